# revision 49
# baseline (speedup 1.0000x reference)
"""Trainium2 Bass kernel for nn_Conv2d_60009283059961.

Single-channel 2D cross-correlation, 8192x8192 image, 7x7 kernel, stride 2,
padding 3, plus scalar bias -> 4096x4096 output.

Strategy (v3: plane-pair partition stacking, 4 accumulation passes)
-------------------------------------------------------------------
Row-shard the output across 8 NeuronCores (512 output rows each).

The padded input is split into even/odd COLUMN planes; a chunk stacks both
planes on the partition axis: partitions 0..62 = even-plane rows, 63..125 =
odd-plane rows of the same 63-row span.  Because padded column sp = 2C + j for
output column C, kernel column j = 2t   reads the even plane at offset C + t
and           kernel column j = 2t + 1 reads the odd  plane at offset C + t:
ONE stationary column offset t serves a PAIR of kernel columns.  The 7 kernel
columns therefore take 4 accumulation passes (t = 0..3; the t=3 odd-plane slot
carries zero band weights) instead of 7 -- PE moving-row count drops from
7*outputs/128 to 4*outputs/128 (~114K -> 65.5K rows, 27.3us at 2.4GHz).

Per (row-block b of 29 out-rows, col-tile c of 128 out-cols, pass t):
  psum[p, m] += sum_k  chunk[k, 128c + t + p] * band[k, 29t + m]
  band[63*plane + r, 29t + m] = w[r - 2m, 2t + plane]
accumulated over t = 0..3 into psum slice [:, 29b : 29b+mb].

The slab is fully resident in SBUF (74KB/partition of the 208KB): 8 column
groups x 18 row-blocks, streamed as ~28 multi-block DMAs (515B descriptors),
emitted just-in-time so each engine sequencer handles its DMA issues in time
order.  All DMA traffic serializes on the ~360GB/s DMA-engine device -- the
roofline here: 9.35MB fp8 input + 2.1MB fp8 output per core ~= 32us busy.
Each group owns 8 PSUM banks as split accumulators (blocks 0..8 / 9..17 per
col-tile) so half the drains retire mid-group; drains (fused bias + 0.4
scale, fp8 cast) alternate between the scalar and vector engines (gpsimd
cannot read PSUM).  Output stores for groups 0..5 are whole-group DMAs on
the sync queue held back by an injected semaphore wait until the input has
transferred (output never preempts input on the shared DMA device); the last
two groups store in halves on parallel queues to shorten the tail.  Output
is stored transposed ([out-col, out-row]) and the host transposes/rescales
for free.  TimelineSim: 37.1us (vs 57.0us for the 7-pass v2, 132us for the
original moving-input formulation).
"""

from contextlib import ExitStack

import numpy as np
import ml_dtypes

import concourse.bass as bass
import concourse.tile as tile
from concourse import mybir
from concourse.bass_utils import run_bass_kernel_spmd

# Problem constants (hardcoded per contract; kernel.py must be self-contained).
H = 8192          # input rows
W = 8192          # input cols
KH = KW = 7
STRIDE = 2
PAD = 3
OH = H // STRIDE  # 4096
OW = W // STRIDE  # 4096
NCORES = 8
RPC = OH // NCORES        # 512 output rows per core

MB = 29                   # output rows per row-block
SPAN = 2 * MB + 5         # 63 input rows per block per plane
KPART = 2 * SPAN          # 126 stationary partitions (both planes)
RSTRIDE = 2 * MB          # 58 input rows between consecutive blocks
NBLOCK = 18               # 17 full blocks + one 19-row block = 512 rows
MB_LAST = RPC - 17 * MB   # 19
NPASS = 4                 # kernel-column pairs (j = 2t, 2t+1)

G = 4                     # col-tiles (128 out-cols each) per group
NGROUPS = OW // (128 * G)  # 8
TILE_W = 128 * G + 3      # 515 plane cols per group chunk
PLANE_W = 4104            # padded plane width (4099 data cols + pad)
NROWS = RSTRIDE * (NBLOCK - 1) + SPAN  # 1049 padded slab rows per plane

BF16 = ml_dtypes.bfloat16
FP8 = ml_dtypes.float8_e3m4
O_DTYPE = "fp8"           # "fp8" (E3M4 output, scaled) or "bf16"
O_SCALE = 0.4             # keeps |out| (max ~38.5) under E3M4's max of 15.5

LAST_RESULTS = None       # test.py introspection hook
LAST_NC = None            # built Bass program, for cost-model timing

# Per-group chunking of the 18 row-blocks into DMAs (group 0 starts with a
# small chunk so the PE can begin ~1us earlier).
CHUNKS_G0 = [1, 2, 2, 3, 3, 3, 4]
CHUNKS_G12 = [6, 4, 4, 4]
CHUNKS_GN = [9, 9]
CHUNKS_GLAST = [6, 6, 3, 3]


def _split_excess_waits(nc, max_waits=1):
    """Workaround: this walrus build allows only one sync wait per
    instruction; spread extra waits across NOPs on the same engine."""
    for fn in nc.m.functions:
        for bb in fn.blocks:
            new = []
            for inst in bb.instructions:
                si = getattr(inst, "sync_info", None)
                if si is not None and si.on_wait is not None and len(si.on_wait) > max_waits:
                    waits = list(si.on_wait)
                    excess, keep = waits[:-max_waits], waits[-max_waits:]
                    for j in range(0, len(excess), max_waits):
                        new.append(mybir.InstNoOp(
                            name=nc.get_next_instruction_name(),
                            sync_info=mybir.SyncInfo(
                                on_wait=excess[j:j + max_waits], on_update=[]),
                            bass_nofuse=True,
                            engine=inst.engine,
                        ))
                    si.on_wait = keep
                new.append(inst)
            bb.instructions[:] = new


def _delay_stores(nc, store_names, last_load_name):
    """Append to each named store DMA a wait on the semaphore the last
    input-chunk DMA updates, at its cumulative value -- holding output
    stores off the shared DMA engines until all input has transferred."""
    if not store_names or last_load_name is None:
        return
    store_names = set(store_names)
    cum = {}
    target = None
    for fn in nc.m.functions:
        for bb in fn.blocks:
            for inst in bb.instructions:
                si = getattr(inst, "sync_info", None)
                if si is None:
                    continue
                for u in (si.on_update or []):
                    key = u.ant_name
                    cum[key] = cum.get(key, 0) + (u.update_value or 0)
                    if inst.name == last_load_name:
                        target = (u, cum[key])
    assert target is not None, "last input load not found"
    u, value = target
    for fn in nc.m.functions:
        for bb in fn.blocks:
            for inst in bb.instructions:
                if inst.name in store_names:
                    si = inst.sync_info
                    w = mybir.SyncWait(
                        sync_type=u.sync_type, id=u.id, ant_name=u.ant_name,
                        wait_mode="sem-ge-imm", wait_value=value)
                    # keep this wait LAST so _split_excess_waits leaves it on
                    # the store itself.
                    si.on_wait = list(si.on_wait or []) + [w]


def _build_program(bias_val: float):
    f32 = mybir.dt.float32
    bf16 = mybir.dt.bfloat16
    xdt = mybir.dt.float8e3
    odt = mybir.dt.float8e3 if O_DTYPE == "fp8" else bf16

    nc = bass.Bass("TRN2", target_bir_lowering=False, debug=False,
                   num_devices=NCORES)
    # Input blocks: [block, partition(plane*63 + r), plane-col].
    x_dram = nc.dram_tensor("xs", [NBLOCK, KPART, PLANE_W], xdt,
                            kind="ExternalInput").ap()
    # Band: [63*plane + r, 29t + m] = w[r - 2m, 2t + plane].
    w_dram = nc.dram_tensor("wb", [KPART, NPASS * MB], bf16,
                            kind="ExternalInput").ap()
    # Transposed output: out_T[n, m] = out[m, n]; host transposes for free.
    out_dram = nc.dram_tensor("out", [OW, RPC], odt, kind="ExternalOutput").ap()

    with tile.TileContext(nc) as tc, ExitStack() as ctx:
        wpool = ctx.enter_context(tc.tile_pool(name="w", bufs=1))
        xpool = ctx.enter_context(tc.tile_pool(name="x", bufs=27))
        opool = ctx.enter_context(tc.tile_pool(name="o", bufs=8))
        ppool = ctx.enter_context(tc.tile_pool(name="p", bufs=4, space="PSUM"))

        w_sb = wpool.tile([KPART, NPASS * MB], bf16)

        # --- input loads ---------------------------------------------------
        # The slab stays resident, but loads are EMITTED just-in-time (group
        # g+2 issued while group g computes) so each engine's SEQ processes
        # its DMA issues in time order: sync (SP) carries nearly all input
        # and has no other duties; the scalar (Act) SEQ must stay free for
        # drains after the first group.
        tail_load_names = []

        def load_chunk(g, b0, nb, eng):
            ch = xpool.tile([KPART, nb, TILE_W], xdt, tag="xchunk")
            src = x_dram[b0:b0 + nb, 0:KPART,
                         128 * G * g: 128 * G * g + TILE_W].rearrange(
                             "b p c -> p b c")
            inst = eng.dma_start(ch[:], src)
            if g >= NGROUPS - 2:
                tail_load_names.append(inst.ins.name)
            return ch

        chunks = {}

        ECYCLE = [nc.sync, nc.scalar, nc.gpsimd]
        ecount = [0]

        def load_group(g):
            if g >= NGROUPS or g in chunks:
                return
            if g == 0:
                sizes = CHUNKS_G0
            elif g <= 2:
                sizes = CHUNKS_G12
            elif g == NGROUPS - 1:
                sizes = CHUNKS_GLAST
            else:
                sizes = CHUNKS_GN
            lst = []
            b0 = 0
            for i, nb in enumerate(sizes):
                if g <= 2:
                    eng = ECYCLE[ecount[0] % 3]
                    ecount[0] += 1
                else:
                    eng = nc.sync
                lst.append((load_chunk(g, b0, nb, eng), b0, nb))
                b0 += nb
            assert b0 == NBLOCK
            chunks[g] = lst

        def chunk_for(g, b):
            for ch, b0, nb in chunks[g]:
                if b0 <= b < b0 + nb:
                    return ch, b - b0
            raise AssertionError

        nc.gpsimd.dma_start(w_sb[:], w_dram[:])
        load_group(0)
        load_group(1)
        load_group(2)

        # --- compute + drain + store --------------------------------------
        # GPSIMD cannot touch PSUM, so drains go on vector (DVE) and scalar
        # (Activation); stores ride gpsimd's SWDGE queue, which is otherwise
        # idle, keeping the two HWDGE queues free for input chunks.
        # Each psum bank is split into two accumulators (blocks 0..8 / 9..17)
        # so half the drain+store volume retires mid-group instead of piling
        # up at the group boundary.
        BSPLIT = 9                  # blocks per first accumulator
        COLS_A = BSPLIT * MB        # 261 psum cols (= out rows) in half A
        COLS_B = RPC - COLS_A       # 251 in half B

        def drain(c, outsb, p, col0, ncols):
            # out = (psum + bias) * O_SCALE; the host multiplies back.
            sc = O_SCALE if O_DTYPE == "fp8" else 1.0
            dst = outsb[0:128, c, col0:col0 + ncols]
            if c % 2 == 0:
                nc.scalar.activation(dst, p[:],
                                     mybir.ActivationFunctionType.Copy,
                                     bias=bias_val * sc, scale=sc)
            elif O_DTYPE == "fp8":
                nc.vector.tensor_scalar(dst, p[:], bias_val, O_SCALE,
                                        mybir.AluOpType.add,
                                        mybir.AluOpType.mult)
            else:
                nc.vector.tensor_scalar_add(dst, p[:], bias_val)

        deferred_stores = []   # (g, outsb) for groups 0..5
        store_names = []       # DMA instruction names needing the input-done wait

        def store(g, outsb, q=None):
            # Groups 0..5: whole-group store [128, 4, 512] on the sync queue,
            # HELD (via an injected sem wait, see _delay_stores) until the
            # input has essentially transferred, so output traffic never
            # delays input delivery.  Groups 6-7 store in halves right away
            # (input is done by then), on parallel queues to shorten the tail.
            if q is None:
                row0, ntiles, t0 = 512 * g, G, 0
                eng = nc.sync
            else:
                row0, ntiles, t0 = 512 * g + 256 * q, 2, 2 * q
                eng = nc.scalar if g < NGROUPS - 1 else nc.sync
            dst = out_dram[row0: row0 + 128 * ntiles, 0:RPC].rearrange(
                "(u p) c -> p u c", p=128)
            inst = eng.dma_start(dst, outsb[0:128, t0:t0 + ntiles, 0:RPC])
            if q is None:
                store_names.append(inst.ins.name)

        for g in range(NGROUPS):
            pA = [ppool.tile([128, COLS_A], f32, tag="accA", name=f"pa{g}_{c}")
                  for c in range(G)]
            pB = [ppool.tile([128, COLS_B], f32, tag="accB", name=f"pb{g}_{c}")
                  for c in range(G)]
            outsb = opool.tile([128, G, RPC], odt, tag="osb")
            # The last group runs as two 2-col-tile half-sweeps over the same
            # chunks: the first half's drains+store retire under the second
            # half's compute, leaving only a 2-tile drain + small store on the
            # kernel's tail.
            csweeps = [range(G)] if g < NGROUPS - 4 else ([(0, 1), (2, 3)] if g < NGROUPS - 1 else [(0, 1), (2,), (3,)])
            for cset in csweeps:
                for b in range(NBLOCK):
                    if b == 1:
                        load_group(g + 3)
                    ch, bi = chunk_for(g, b)
                    mb = MB if b < NBLOCK - 1 else MB_LAST
                    half, bb = (pA, b) if b < BSPLIT else (pB, b - BSPLIT)
                    for c in cset:
                        for t in range(NPASS):
                            off = 128 * c + t
                            nc.tensor.matmul(
                                half[c][0:128, MB * bb: MB * bb + mb],
                                ch[0:KPART, bi:bi + 1, off:off + 128],
                                w_sb[0:KPART, MB * t: MB * t + mb],
                                start=(t == 0), stop=(t == NPASS - 1))
                        if b == BSPLIT - 1:
                            drain(c, outsb, pA[c], 0, COLS_A)
                        elif b == NBLOCK - 1:
                            drain(c, outsb, pB[c], COLS_A, COLS_B)
                            if g >= NGROUPS - 2:
                                if c % 2 == 1:
                                    store(g, outsb, q=c // 2)
                            elif c == G - 1:
                                deferred_stores.append((g, outsb))
            if g == NGROUPS - 1:
                for gg, osb in deferred_stores:
                    store(gg, osb)

    _delay_stores(nc, store_names, tail_load_names[-5])
    _split_excess_waits(nc)
    return nc


def kernel(enc_x, weight, bias, num_row, num_col):
    global LAST_RESULTS, LAST_NC
    enc_x = np.asarray(enc_x, dtype=np.float32)
    weight = np.asarray(weight, dtype=np.float32).reshape(KH, KW)
    bias_val = float(np.asarray(bias).reshape(-1)[0])
    assert int(num_row) == H and int(num_col) == W

    x = enc_x.reshape(H, W)

    # Band: wband[63*plane + r, 29t + m] = w[r - 2m, 2t + plane].
    wband = np.zeros((KPART, NPASS * MB), dtype=np.float32)
    for t in range(NPASS):
        for plane in range(2):
            j = 2 * t + plane
            if j >= KW:
                continue
            for m in range(MB):
                for i in range(KH):
                    r = 2 * m + i
                    wband[SPAN * plane + r, MB * t + m] = weight[i, j]
    wband = wband.astype(BF16)

    # Per-core input slabs: pad, split into even/odd column planes, then
    # gather overlapping 63-row blocks with both planes stacked on the
    # partition axis.  Core c computes output rows [512c, 512c+512); its
    # slab local row li <-> global row 1024c - 3 + li; padded col sp = s + 3.
    in_maps = []
    for core in range(NCORES):
        xp = np.zeros((NROWS, 2 * PLANE_W), dtype=np.float32)
        g0 = 1024 * core - 3
        src_lo = max(0, g0)
        src_hi = min(H, g0 + NROWS)
        xp[src_lo - g0:src_hi - g0, 3:3 + W] = x[src_lo:src_hi, :]
        planes = np.stack([xp[:, 0::2], xp[:, 1::2]])  # [2, NROWS, PLANE_W]
        s0, s1, s2 = planes.strides
        blk = np.lib.stride_tricks.as_strided(
            planes, shape=(NBLOCK, 2, SPAN, PLANE_W),
            strides=(RSTRIDE * s1, s0, s1, s2))
        xs = np.ascontiguousarray(blk).reshape(NBLOCK, KPART, PLANE_W)
        in_maps.append({"xs": xs.astype(FP8), "wb": wband})

    nc = _build_program(bias_val)
    LAST_NC = nc
    try:
        res = run_bass_kernel_spmd(nc, in_maps, core_ids=list(range(NCORES)))
    except ModuleNotFoundError:
        # BASS_TRACE was requested but this environment lacks the axon NTFF
        # profile hook; rerun untraced.
        import os
        os.environ["BASS_NEVER_TRACE"] = "1"
        res = run_bass_kernel_spmd(nc, in_maps, core_ids=list(range(NCORES)))
    LAST_RESULTS = res

    # Gather: each core returns out_T [4096, 512]; transpose + stack, undo
    # the on-device O_SCALE.
    unscale = 1.0 / O_SCALE if O_DTYPE == "fp8" else 1.0
    out = np.concatenate(
        [np.asarray(res.results[c]["out"]).astype(np.float32).T * unscale
         for c in range(NCORES)], axis=0)
    return out.reshape(-1)


# revision 50
# speedup vs baseline: 1.0004x; 1.0004x over previous
"""Trainium2 Bass kernel for nn_Conv2d_60009283059961.

Single-channel 2D cross-correlation, 8192x8192 image, 7x7 kernel, stride 2,
padding 3, plus scalar bias -> 4096x4096 output.

Strategy (v3: plane-pair partition stacking, 4 accumulation passes)
-------------------------------------------------------------------
Row-shard the output across 8 NeuronCores (512 output rows each).

The padded input is split into even/odd COLUMN planes; a chunk stacks both
planes on the partition axis: partitions 0..62 = even-plane rows, 63..125 =
odd-plane rows of the same 63-row span.  Because padded column sp = 2C + j for
output column C, kernel column j = 2t   reads the even plane at offset C + t
and           kernel column j = 2t + 1 reads the odd  plane at offset C + t:
ONE stationary column offset t serves a PAIR of kernel columns.  The 7 kernel
columns therefore take 4 accumulation passes (t = 0..3; the t=3 odd-plane slot
carries zero band weights) instead of 7 -- PE moving-row count drops from
7*outputs/128 to 4*outputs/128 (~114K -> 65.5K rows, 27.3us at 2.4GHz).

Per (row-block b of 29 out-rows, col-tile c of 128 out-cols, pass t):
  psum[p, m] += sum_k  chunk[k, 128c + t + p] * band[k, 29t + m]
  band[63*plane + r, 29t + m] = w[r - 2m, 2t + plane]
accumulated over t = 0..3 into psum slice [:, 29b : 29b+mb].

The slab is fully resident in SBUF (74KB/partition of the 208KB): 8 column
groups x 18 row-blocks, streamed as ~28 multi-block DMAs (515B descriptors),
emitted just-in-time so each engine sequencer handles its DMA issues in time
order.  All DMA traffic serializes on the ~360GB/s DMA-engine device -- the
roofline here: 9.35MB fp8 input + 2.1MB fp8 output per core ~= 32us busy.
Each group owns 8 PSUM banks as split accumulators (blocks 0..8 / 9..17 per
col-tile) so half the drains retire mid-group; drains (fused bias + 0.4
scale, fp8 cast) alternate between the scalar and vector engines (gpsimd
cannot read PSUM).  Output stores for groups 0..5 are whole-group DMAs on
the sync queue held back by an injected semaphore wait until the input has
transferred (output never preempts input on the shared DMA device); the last
two groups store in halves on parallel queues to shorten the tail.  Output
is stored transposed ([out-col, out-row]) and the host transposes/rescales
for free.  TimelineSim: 37.1us (vs 57.0us for the 7-pass v2, 132us for the
original moving-input formulation).
"""

from contextlib import ExitStack

import numpy as np
import ml_dtypes

import concourse.bass as bass
import concourse.tile as tile
from concourse import mybir
from concourse.bass_utils import run_bass_kernel_spmd

# Problem constants (hardcoded per contract; kernel.py must be self-contained).
H = 8192          # input rows
W = 8192          # input cols
KH = KW = 7
STRIDE = 2
PAD = 3
OH = H // STRIDE  # 4096
OW = W // STRIDE  # 4096
NCORES = 8
RPC = OH // NCORES        # 512 output rows per core

MB = 29                   # output rows per row-block
SPAN = 2 * MB + 5         # 63 input rows per block per plane
KPART = 2 * SPAN          # 126 stationary partitions (both planes)
RSTRIDE = 2 * MB          # 58 input rows between consecutive blocks
NBLOCK = 18               # 17 full blocks + one 19-row block = 512 rows
MB_LAST = RPC - 17 * MB   # 19
NPASS = 4                 # kernel-column pairs (j = 2t, 2t+1)

G = 4                     # col-tiles (128 out-cols each) per group
NGROUPS = OW // (128 * G)  # 8
TILE_W = 128 * G + 3      # 515 plane cols per group chunk
PLANE_W = 4104            # padded plane width (4099 data cols + pad)
NROWS = RSTRIDE * (NBLOCK - 1) + SPAN  # 1049 padded slab rows per plane

BF16 = ml_dtypes.bfloat16
FP8 = ml_dtypes.float8_e3m4
O_DTYPE = "fp8"           # "fp8" (E3M4 output, scaled) or "bf16"
O_SCALE = 0.4             # keeps |out| (max ~38.5) under E3M4's max of 15.5

LAST_RESULTS = None       # test.py introspection hook
LAST_NC = None            # built Bass program, for cost-model timing

# Per-group chunking of the 18 row-blocks into DMAs (group 0 starts with a
# small chunk so the PE can begin ~1us earlier).
CHUNKS_G0 = [1, 2, 2, 3, 3, 3, 4]
CHUNKS_G12 = [6, 4, 4, 4]
CHUNKS_GN = [10, 8]
CHUNKS_GLAST = [6, 6, 3, 3]


def _split_excess_waits(nc, max_waits=1):
    """Workaround: this walrus build allows only one sync wait per
    instruction; spread extra waits across NOPs on the same engine."""
    for fn in nc.m.functions:
        for bb in fn.blocks:
            new = []
            for inst in bb.instructions:
                si = getattr(inst, "sync_info", None)
                if si is not None and si.on_wait is not None and len(si.on_wait) > max_waits:
                    waits = list(si.on_wait)
                    excess, keep = waits[:-max_waits], waits[-max_waits:]
                    for j in range(0, len(excess), max_waits):
                        new.append(mybir.InstNoOp(
                            name=nc.get_next_instruction_name(),
                            sync_info=mybir.SyncInfo(
                                on_wait=excess[j:j + max_waits], on_update=[]),
                            bass_nofuse=True,
                            engine=inst.engine,
                        ))
                    si.on_wait = keep
                new.append(inst)
            bb.instructions[:] = new


def _delay_stores(nc, store_names, last_load_name):
    """Append to each named store DMA a wait on the semaphore the last
    input-chunk DMA updates, at its cumulative value -- holding output
    stores off the shared DMA engines until all input has transferred."""
    if not store_names or last_load_name is None:
        return
    store_names = set(store_names)
    cum = {}
    target = None
    for fn in nc.m.functions:
        for bb in fn.blocks:
            for inst in bb.instructions:
                si = getattr(inst, "sync_info", None)
                if si is None:
                    continue
                for u in (si.on_update or []):
                    key = u.ant_name
                    cum[key] = cum.get(key, 0) + (u.update_value or 0)
                    if inst.name == last_load_name:
                        target = (u, cum[key])
    assert target is not None, "last input load not found"
    u, value = target
    for fn in nc.m.functions:
        for bb in fn.blocks:
            for inst in bb.instructions:
                if inst.name in store_names:
                    si = inst.sync_info
                    w = mybir.SyncWait(
                        sync_type=u.sync_type, id=u.id, ant_name=u.ant_name,
                        wait_mode="sem-ge-imm", wait_value=value)
                    # keep this wait LAST so _split_excess_waits leaves it on
                    # the store itself.
                    si.on_wait = list(si.on_wait or []) + [w]


def _build_program(bias_val: float):
    f32 = mybir.dt.float32
    bf16 = mybir.dt.bfloat16
    xdt = mybir.dt.float8e3
    odt = mybir.dt.float8e3 if O_DTYPE == "fp8" else bf16

    nc = bass.Bass("TRN2", target_bir_lowering=False, debug=False,
                   num_devices=NCORES)
    # Input blocks: [block, partition(plane*63 + r), plane-col].
    x_dram = nc.dram_tensor("xs", [NBLOCK, KPART, PLANE_W], xdt,
                            kind="ExternalInput").ap()
    # Band: [63*plane + r, 29t + m] = w[r - 2m, 2t + plane].
    w_dram = nc.dram_tensor("wb", [KPART, NPASS * MB], bf16,
                            kind="ExternalInput").ap()
    # Transposed output: out_T[n, m] = out[m, n]; host transposes for free.
    out_dram = nc.dram_tensor("out", [OW, RPC], odt, kind="ExternalOutput").ap()

    with tile.TileContext(nc) as tc, ExitStack() as ctx:
        wpool = ctx.enter_context(tc.tile_pool(name="w", bufs=1))
        xpool = ctx.enter_context(tc.tile_pool(name="x", bufs=27))
        opool = ctx.enter_context(tc.tile_pool(name="o", bufs=8))
        ppool = ctx.enter_context(tc.tile_pool(name="p", bufs=4, space="PSUM"))

        w_sb = wpool.tile([KPART, NPASS * MB], bf16)

        # --- input loads ---------------------------------------------------
        # The slab stays resident, but loads are EMITTED just-in-time (group
        # g+2 issued while group g computes) so each engine's SEQ processes
        # its DMA issues in time order: sync (SP) carries nearly all input
        # and has no other duties; the scalar (Act) SEQ must stay free for
        # drains after the first group.
        tail_load_names = []

        def load_chunk(g, b0, nb, eng):
            ch = xpool.tile([KPART, nb, TILE_W], xdt, tag="xchunk")
            src = x_dram[b0:b0 + nb, 0:KPART,
                         128 * G * g: 128 * G * g + TILE_W].rearrange(
                             "b p c -> p b c")
            inst = eng.dma_start(ch[:], src)
            if g >= NGROUPS - 2:
                tail_load_names.append(inst.ins.name)
            return ch

        chunks = {}

        ECYCLE = [nc.sync, nc.scalar, nc.gpsimd]
        ecount = [0]

        def load_group(g):
            if g >= NGROUPS or g in chunks:
                return
            if g == 0:
                sizes = CHUNKS_G0
            elif g <= 2:
                sizes = CHUNKS_G12
            elif g == NGROUPS - 1:
                sizes = CHUNKS_GLAST
            else:
                sizes = CHUNKS_GN
            lst = []
            b0 = 0
            for i, nb in enumerate(sizes):
                if g <= 2:
                    eng = ECYCLE[ecount[0] % 3]
                    ecount[0] += 1
                else:
                    eng = nc.sync
                lst.append((load_chunk(g, b0, nb, eng), b0, nb))
                b0 += nb
            assert b0 == NBLOCK
            chunks[g] = lst

        def chunk_for(g, b):
            for ch, b0, nb in chunks[g]:
                if b0 <= b < b0 + nb:
                    return ch, b - b0
            raise AssertionError

        nc.gpsimd.dma_start(w_sb[:], w_dram[:])
        load_group(0)
        load_group(1)
        load_group(2)

        # --- compute + drain + store --------------------------------------
        # GPSIMD cannot touch PSUM, so drains go on vector (DVE) and scalar
        # (Activation); stores ride gpsimd's SWDGE queue, which is otherwise
        # idle, keeping the two HWDGE queues free for input chunks.
        # Each psum bank is split into two accumulators (blocks 0..8 / 9..17)
        # so half the drain+store volume retires mid-group instead of piling
        # up at the group boundary.
        BSPLIT = 9                  # blocks per first accumulator
        COLS_A = BSPLIT * MB        # 261 psum cols (= out rows) in half A
        COLS_B = RPC - COLS_A       # 251 in half B

        def drain(c, outsb, p, col0, ncols):
            # out = (psum + bias) * O_SCALE; the host multiplies back.
            sc = O_SCALE if O_DTYPE == "fp8" else 1.0
            dst = outsb[0:128, c, col0:col0 + ncols]
            if c % 2 == 0:
                nc.scalar.activation(dst, p[:],
                                     mybir.ActivationFunctionType.Copy,
                                     bias=bias_val * sc, scale=sc)
            elif O_DTYPE == "fp8":
                nc.vector.tensor_scalar(dst, p[:], bias_val, O_SCALE,
                                        mybir.AluOpType.add,
                                        mybir.AluOpType.mult)
            else:
                nc.vector.tensor_scalar_add(dst, p[:], bias_val)

        deferred_stores = []   # (g, outsb) for groups 0..5
        store_names = []       # DMA instruction names needing the input-done wait

        def store(g, outsb, q=None):
            # Groups 0..5: whole-group store [128, 4, 512] on the sync queue,
            # HELD (via an injected sem wait, see _delay_stores) until the
            # input has essentially transferred, so output traffic never
            # delays input delivery.  Groups 6-7 store in halves right away
            # (input is done by then), on parallel queues to shorten the tail.
            if q is None:
                row0, ntiles, t0 = 512 * g, G, 0
                eng = nc.sync
            else:
                row0, ntiles, t0 = 512 * g + 256 * q, 2, 2 * q
                eng = nc.scalar if g < NGROUPS - 1 else nc.sync
            dst = out_dram[row0: row0 + 128 * ntiles, 0:RPC].rearrange(
                "(u p) c -> p u c", p=128)
            inst = eng.dma_start(dst, outsb[0:128, t0:t0 + ntiles, 0:RPC])
            if q is None:
                store_names.append(inst.ins.name)

        for g in range(NGROUPS):
            pA = [ppool.tile([128, COLS_A], f32, tag="accA", name=f"pa{g}_{c}")
                  for c in range(G)]
            pB = [ppool.tile([128, COLS_B], f32, tag="accB", name=f"pb{g}_{c}")
                  for c in range(G)]
            outsb = opool.tile([128, G, RPC], odt, tag="osb")
            # The last group runs as two 2-col-tile half-sweeps over the same
            # chunks: the first half's drains+store retire under the second
            # half's compute, leaving only a 2-tile drain + small store on the
            # kernel's tail.
            csweeps = [range(G)] if g < NGROUPS - 4 else ([(0, 1), (2, 3)] if g < NGROUPS - 1 else [(0, 1), (2,), (3,)])
            for cset in csweeps:
                for b in range(NBLOCK):
                    if b == 1:
                        load_group(g + 3)
                    ch, bi = chunk_for(g, b)
                    mb = MB if b < NBLOCK - 1 else MB_LAST
                    half, bb = (pA, b) if b < BSPLIT else (pB, b - BSPLIT)
                    for c in cset:
                        for t in range(NPASS):
                            off = 128 * c + t
                            nc.tensor.matmul(
                                half[c][0:128, MB * bb: MB * bb + mb],
                                ch[0:KPART, bi:bi + 1, off:off + 128],
                                w_sb[0:KPART, MB * t: MB * t + mb],
                                start=(t == 0), stop=(t == NPASS - 1))
                        if b == BSPLIT - 1:
                            drain(c, outsb, pA[c], 0, COLS_A)
                        elif b == NBLOCK - 1:
                            drain(c, outsb, pB[c], COLS_A, COLS_B)
                            if g >= NGROUPS - 2:
                                if c % 2 == 1:
                                    store(g, outsb, q=c // 2)
                            elif c == G - 1:
                                deferred_stores.append((g, outsb))
            if g == NGROUPS - 1:
                for gg, osb in deferred_stores:
                    store(gg, osb)

    _delay_stores(nc, store_names, tail_load_names[-5])
    _split_excess_waits(nc)
    return nc


def kernel(enc_x, weight, bias, num_row, num_col):
    global LAST_RESULTS, LAST_NC
    enc_x = np.asarray(enc_x, dtype=np.float32)
    weight = np.asarray(weight, dtype=np.float32).reshape(KH, KW)
    bias_val = float(np.asarray(bias).reshape(-1)[0])
    assert int(num_row) == H and int(num_col) == W

    x = enc_x.reshape(H, W)

    # Band: wband[63*plane + r, 29t + m] = w[r - 2m, 2t + plane].
    wband = np.zeros((KPART, NPASS * MB), dtype=np.float32)
    for t in range(NPASS):
        for plane in range(2):
            j = 2 * t + plane
            if j >= KW:
                continue
            for m in range(MB):
                for i in range(KH):
                    r = 2 * m + i
                    wband[SPAN * plane + r, MB * t + m] = weight[i, j]
    wband = wband.astype(BF16)

    # Per-core input slabs: pad, split into even/odd column planes, then
    # gather overlapping 63-row blocks with both planes stacked on the
    # partition axis.  Core c computes output rows [512c, 512c+512); its
    # slab local row li <-> global row 1024c - 3 + li; padded col sp = s + 3.
    in_maps = []
    for core in range(NCORES):
        xp = np.zeros((NROWS, 2 * PLANE_W), dtype=np.float32)
        g0 = 1024 * core - 3
        src_lo = max(0, g0)
        src_hi = min(H, g0 + NROWS)
        xp[src_lo - g0:src_hi - g0, 3:3 + W] = x[src_lo:src_hi, :]
        planes = np.stack([xp[:, 0::2], xp[:, 1::2]])  # [2, NROWS, PLANE_W]
        s0, s1, s2 = planes.strides
        blk = np.lib.stride_tricks.as_strided(
            planes, shape=(NBLOCK, 2, SPAN, PLANE_W),
            strides=(RSTRIDE * s1, s0, s1, s2))
        xs = np.ascontiguousarray(blk).reshape(NBLOCK, KPART, PLANE_W)
        in_maps.append({"xs": xs.astype(FP8), "wb": wband})

    nc = _build_program(bias_val)
    LAST_NC = nc
    try:
        res = run_bass_kernel_spmd(nc, in_maps, core_ids=list(range(NCORES)))
    except ModuleNotFoundError:
        # BASS_TRACE was requested but this environment lacks the axon NTFF
        # profile hook; rerun untraced.
        import os
        os.environ["BASS_NEVER_TRACE"] = "1"
        res = run_bass_kernel_spmd(nc, in_maps, core_ids=list(range(NCORES)))
    LAST_RESULTS = res

    # Gather: each core returns out_T [4096, 512]; transpose + stack, undo
    # the on-device O_SCALE.
    unscale = 1.0 / O_SCALE if O_DTYPE == "fp8" else 1.0
    out = np.concatenate(
        [np.asarray(res.results[c]["out"]).astype(np.float32).T * unscale
         for c in range(NCORES)], axis=0)
    return out.reshape(-1)
